# revision 28
# baseline (speedup 1.0000x reference)
"""KAARMA-style multi-cell kernel recurrence on 8 Trainium2 NeuronCores.

Math reformulation (validated vs reference to ~5e-3 rel in bf16):
  per step t, per batch b, for every dictionary atom (c, m) [cm = 800 atoms]:
    phi[b,cm]  = exp(-|s_b - S_cm|^2 - (x_tb - U_cm)^2)
    gate[b,c]  = softmax_c(MLP(x_tb))               (precomputable, x-only)
    s'_b       = sum_cm gate[b,cell(cm)] * phi[b,cm] * A[cm,:]
  Expand the squares and fold the gate into the exponent:
    psi[b,cm]  = exp( 2 s_b.S_cm - |s_b|^2 + 2 x U_cm - x^2 + logsoftmax_c )
    s'_b       = sum_cm psi[b,cm] * (A[cm,:] * exp(-|S_cm|^2 - U_cm^2))
  so one step = matmul([K,B] -> [896,B]) -> exp -> matmul([896,B] -> [16,B]).

The wall clock is bound by the per-step serial chain
  mm1 -> exp -> mm2 -> state copy(+square) -> mm1
so the design minimizes chain latency: bf16 matmuls (1 cyc/row vs 4 for
fp32), ONE exp instruction per stream-step, and n_streams independent
batch slices whose chains interleave on the engines.  All per-step side
work (x-row staging, output extraction) is hoisted off the chain into
per-stage DMAs against a stage-wide rhs tile:

Contraction-row layout of the stage tile R [53, STAGE*BH] (engine SBUF
writes must start at partition 0/32/64/96; DMA writes are unrestricted,
so DMA'd x-rows fill the alignment gaps):
  rows  0:16  s^2      (weights -1)                <- Pool square (SBUF)
  rows 16:32  x-rows A (DMA, precomputed on host)
  rows 32:48  s        (weights 2*S in bf16)      <- DVE copy from PSUM
  rows 48:53  x-rows B (DMA)
The handoff obeys three BIR-verifier rules found the hard way: GPSIMD
cannot touch PSUM; DVE/ACT may read at most ONE PSUM operand per
instruction; two SBUF inputs must share a base partition.  So the DVE
does the single PSUM read (copy s), and the square runs on the Pool
engine from the fresh SBUF copy (same-AP inputs), which keeps the DVE
queue short and costs one extra cross-engine hop off the critical copy.

x-rows use hi/lo bf16 splits so only the state path rounds to bf16:
  A: x_hi(w=2U_hi), x_lo(w=2U_hi), x_hi(w=2U_lo), x2_hi(w=-1),
     x2_lo(w=-1), lg_hi c0..7 (one-hot), lg_lo c0..2 (one-hot)
  B: lg_lo c3..7 (one-hot)
Row 32 of R doubles as the output staging area (state component N-1 is
permuted to row 0 of s): one DMA per stage reads it back, shifted one
block (R block ti holds the state ENTERING step ti = output of ti-1).

Sharding: pure data parallel, batch 512 -> 64 per core on 8 cores,
4 streams x 16 batch per core.  Timing (TimelineSim, per step): the
serial chain is ~1516 ns [exp 278+185 drain | mm2 42+173 | copy 142+125
| pool-sq 127 | mm1 49+173 | plus ~40ns/edge sem hops]; measured period
~1680 ns/step vs 2205 for the fp32 baseline.
"""

from collections import deque

import numpy as np

N_CORES = 8
CM = 800
CM_PAD = 896
NCHUNK = 7
KROWS = 53
NSTATE = 16
SOUT = 16  # mm2 output rows (permuted state)
STAGE = 64
NSTREAM = 4
NXROW = 21  # host-precomputed x rows per stream (16 + 5 around the s rows)
MM2_LAG = 2  # stream-slots between exp issue and its mm2 issue
MM2_F32 = False  # fp32 mm2: no Ldweights per matmul (self-loading), 4 cyc/row
PSI_BUFS = 3
R_BUFS = 4
OP2_POOL = True
STAGGER_COLS = 0  # startup phase-stagger dummy work per stream

_PROGRAM_CACHE = {}


def _build_program(B_local, T):
    import concourse.bass as bass
    import concourse.bacc as bacc
    import concourse.tile as tile
    from concourse import mybir
    from contextlib import ExitStack

    f32 = mybir.dt.float32
    bf16 = mybir.dt.bfloat16
    Act = mybir.ActivationFunctionType

    BH = B_local // NSTREAM
    SCOLS = STAGE * BH
    NSTAGE = T // STAGE
    PCOLS = NCHUNK * BH

    # Bacc (not Bass): its compile() runs generate_event_semaphores, which
    # splits multi-wait instructions (TRN2 allows 1 wait per instruction)
    nc = bacc.Bacc("TRN2", target_bir_lowering=False, debug=False)
    W_d = nc.dram_tensor("Wk", [KROWS, CM_PAD], bf16, kind="ExternalInput")
    mm2_dt = f32 if MM2_F32 else bf16
    A2_d = nc.dram_tensor("A2e", [CM_PAD, SOUT], mm2_dt, kind="ExternalInput")
    R_d = nc.dram_tensor("Rt", [NSTREAM * NXROW, T * BH], bf16, kind="ExternalInput")
    O_d = nc.dram_tensor("O1", [NSTREAM, T * BH], bf16, kind="ExternalOutput")

    with tile.TileContext(nc) as tc, ExitStack() as ctx:
        singles = ctx.enter_context(tc.tile_pool(name="singles", bufs=1))
        rpool = ctx.enter_context(tc.tile_pool(name="rstage", bufs=R_BUFS))
        psipool = ctx.enter_context(tc.tile_pool(name="psi", bufs=PSI_BUFS))
        scrpool = ctx.enter_context(tc.tile_pool(name="scr", bufs=2))
        apsum = ctx.enter_context(tc.tile_pool(name="apsum", bufs=1, space="PSUM"))
        spsum = ctx.enter_context(tc.tile_pool(name="spsum", bufs=1, space="PSUM"))

        Wsb = singles.tile([KROWS, CM_PAD], bf16)
        nc.sync.dma_start(out=Wsb, in_=W_d[:, :])
        A2sb = singles.tile([128, NCHUNK, SOUT], mm2_dt)
        nc.sync.dma_start(out=A2sb, in_=A2_d.rearrange("(c p) n -> p c n", p=128))

        Rt = {}  # si -> [tile per stream]

        def alloc_stage(si):
            tiles = []
            for s in range(NSTREAM):
                rt = rpool.tile([KROWS, SCOLS], bf16, tag=f"r{s}", name=f"r{s}_{si}")
                if si < NSTAGE:
                    lo, hi = si * SCOLS, (si + 1) * SCOLS
                    nc.sync.dma_start(
                        out=rt[16:32, :], in_=R_d[s * NXROW : s * NXROW + 16, lo:hi]
                    )
                    nc.sync.dma_start(
                        out=rt[48:KROWS, :],
                        in_=R_d[s * NXROW + 16 : (s + 1) * NXROW, lo:hi],
                    )
                tiles.append(rt)
            Rt[si] = tiles

        ROW_OUT = 32  # s row 0 (output component) lives at partition 32

        def issue_odma(si):
            # R row 48, block ti = state entering step si*STAGE+ti = output of
            # the previous step, so the stage's outputs are shifted one block.
            for s in range(NSTREAM):
                rt = Rt[si][s]
                if si == 0:
                    nc.sync.dma_start(
                        out=O_d[s : s + 1, 0 : (STAGE - 1) * BH],
                        in_=rt[ROW_OUT : ROW_OUT + 1, BH:SCOLS],
                    )
                else:
                    base = (si * STAGE - 1) * BH
                    nc.sync.dma_start(
                        out=O_d[s : s + 1, base : base + SCOLS],
                        in_=rt[ROW_OUT : ROW_OUT + 1, :],
                    )

        alloc_stage(0)
        for s in range(NSTREAM):
            if s > 0 and STAGGER_COLS > 0:
                # dummy Pool work: staggers stream phases at startup so the
                # steady state spreads exps ~C/4 apart instead of convoying
                nc.gpsimd.memset(Rt[0][s][0:16, 0 : STAGGER_COLS], 0.0)
            nc.gpsimd.memset(Rt[0][s][0:16, 0:BH], 0.0)
            nc.gpsimd.memset(Rt[0][s][32:48, 0:BH], 0.0)
        alloc_stage(1)

        sP_val = [None] * NSTREAM
        psi_val = [None] * NSTREAM
        pend = deque()

        def issue_mm2(s, t):
            sp = spsum.tile([SOUT, BH], f32, tag=f"s{s}", name=f"sp{s}_{t}")
            psi = psi_val[s]
            for k in range(NCHUNK):
                nc.tensor.matmul(
                    sp,
                    lhsT=A2sb[:, k, :],
                    rhs=psi[:, k * BH : (k + 1) * BH],
                    start=(k == 0),
                    stop=(k == NCHUNK - 1),
                    skip_group_check=True,
                )
            sP_val[s] = sp

        for t in range(T):
            si, ti = divmod(t, STAGE)
            if ti == 0 and si > 0:
                alloc_stage(si + 1)
                issue_odma(si - 1)
            for s in range(NSTREAM):
                rt = Rt[si][s]
                col = ti * BH
                if len(pend) >= min(MM2_LAG, NSTREAM - 1) + 1:
                    issue_mm2(*pend.popleft())
                if t > 0:
                    # state handoff: DVE copy (the one legal PSUM read) into
                    # the s rows, then a same-AP square into the s^2 rows
                    # (base partitions 32 and 0 are both write-legal).
                    nc.vector.tensor_scalar_add(
                        rt[32:48, col : col + BH], sP_val[s], 0.0
                    )
                    sq_eng = nc.gpsimd if OP2_POOL else nc.vector
                    sq_eng.tensor_mul(
                        rt[0:16, col : col + BH],
                        rt[32:48, col : col + BH],
                        rt[32:48, col : col + BH],
                    )
                arg = apsum.tile([128, PCOLS], f32, tag=f"a{s}", name=f"arg{s}_{t}")
                for k in range(NCHUNK):
                    nc.tensor.matmul(
                        arg[:, k * BH : (k + 1) * BH],
                        lhsT=Wsb[:, k * 128 : (k + 1) * 128],
                        rhs=rt[0:KROWS, col : col + BH],
                        start=True,
                        stop=True,
                    )
                psi = psipool.tile([128, PCOLS], mm2_dt, tag=f"p{s}", name=f"psi{s}_{t}")
                nc.scalar.activation(out=psi, in_=arg, func=Act.Exp)
                psi_val[s] = psi
                pend.append((s, t))

        while pend:
            issue_mm2(*pend.popleft())
        # final states -> phantom stage block 0 (row 32 = last outputs)
        for s in range(NSTREAM):
            nc.vector.tensor_scalar_add(Rt[NSTAGE][s][32:48, 0:BH], sP_val[s], 0.0)
        issue_odma(NSTAGE - 1)
        for s in range(NSTREAM):
            nc.sync.dma_start(
                out=O_d[s : s + 1, (T - 1) * BH : T * BH],
                in_=Rt[NSTAGE][s][ROW_OUT : ROW_OUT + 1, 0:BH],
            )

    nc.compile()
    return nc


def _host_precompute(x, S, U, A, W1, b1, W2, b2):
    import ml_dtypes

    BF = ml_dtypes.bfloat16

    def rb(v):
        return np.asarray(v, BF).astype(np.float32)

    B, T = x.shape
    C, M, N = S.shape
    B_local = B // N_CORES
    BH = B_local // NSTREAM

    # state permutation: put the output component (N-1) at row 0
    perm = np.r_[N - 1, np.arange(N - 1)]

    Sf = S.reshape(C * M, N).astype(np.float64)
    Uf = U.reshape(C * M).astype(np.float64)
    C1 = (Sf * Sf).sum(1) + Uf * Uf
    A2e = np.zeros((CM_PAD, N), np.float32)
    A2e[:CM] = ((A.reshape(C * M, N) * np.exp(-C1)[:, None])[:, perm]).astype(
        np.float32
    )
    twoU = 2.0 * Uf
    twoU_hi = rb(twoU)
    twoU_lo = rb(twoU - twoU_hi)

    Wk = np.zeros((KROWS, CM_PAD), np.float32)
    Wk[0:16, :CM] = -1.0  # s^2 rows
    Wk[16, :CM] = twoU_hi  # x_hi
    Wk[17, :CM] = twoU_hi  # x_lo
    Wk[18, :CM] = twoU_lo  # x_hi (again)
    Wk[19, :CM] = -1.0  # x2_hi
    Wk[20, :CM] = -1.0  # x2_lo
    for c in range(C):
        Wk[21 + c, c * M : (c + 1) * M] = 1.0  # lg_hi
    for c in range(3):
        Wk[29 + c, c * M : (c + 1) * M] = 1.0  # lg_lo c0..2
    Wk[32:48, :CM] = rb(2.0 * Sf.T[perm])  # s rows
    for c in range(3, 8):
        Wk[48 + c - 3, c * M : (c + 1) * M] = 1.0  # lg_lo c3..7

    # gate log-softmax, x-only (fp32 on host)
    h = np.maximum(x[..., None] * W1[0] + b1, 0.0)  # [B,T,16]
    g = h @ W2 + b2  # [B,T,C]
    g = g - g.max(-1, keepdims=True)
    lg = (g - np.log(np.exp(g).sum(-1, keepdims=True))).astype(np.float32)

    x_hi = rb(x)
    x_lo = rb(x - x_hi)
    x2 = (x.astype(np.float64) ** 2).astype(np.float32)
    x2_hi = rb(x2)
    x2_lo = rb(x2 - x2_hi)
    lg_hi = rb(lg)
    lg_lo = rb(lg - lg_hi)

    # R rows per (core, stream): [21, T, BH] -> [21, T*BH]
    R = np.zeros((N_CORES, NSTREAM, NXROW, T, BH), np.float32)
    for i in range(N_CORES):
        for s in range(NSTREAM):
            bs = slice(i * B_local + s * BH, i * B_local + (s + 1) * BH)
            R[i, s, 0] = x_hi[bs].T
            R[i, s, 1] = x_lo[bs].T
            R[i, s, 2] = x_hi[bs].T
            R[i, s, 3] = x2_hi[bs].T
            R[i, s, 4] = x2_lo[bs].T
            R[i, s, 5:13] = lg_hi[bs].transpose(2, 1, 0)
            R[i, s, 13:16] = lg_lo[bs, :, 0:3].transpose(2, 1, 0)
            R[i, s, 16:21] = lg_lo[bs, :, 3:8].transpose(2, 1, 0)
    R = R.reshape(N_CORES, NSTREAM * NXROW, T * BH)

    Wk_b = Wk.astype(BF)
    A2_b = A2e.astype(np.float32 if MM2_F32 else BF)
    R_b = R.astype(BF)
    return Wk_b, A2_b, R_b


def kernel(x, S, U, A, W1, b1, W2, b2):
    x = np.asarray(x, np.float32)
    B, T = x.shape
    assert B % (N_CORES * NSTREAM) == 0 and T % STAGE == 0
    B_local = B // N_CORES
    BH = B_local // NSTREAM

    Wk, A2e, R = _host_precompute(
        np.asarray(x), np.asarray(S), np.asarray(U), np.asarray(A),
        np.asarray(W1), np.asarray(b1), np.asarray(W2), np.asarray(b2),
    )

    key = (B_local, T)
    if key not in _PROGRAM_CACHE:
        _PROGRAM_CACHE[key] = _build_program(B_local, T)
    nc = _PROGRAM_CACHE[key]

    from concourse.bass_utils import run_bass_kernel_spmd

    in_maps = [
        {"Wk": Wk, "A2e": A2e, "Rt": np.ascontiguousarray(R[i])}
        for i in range(N_CORES)
    ]
    res = run_bass_kernel_spmd(nc, in_maps, core_ids=list(range(N_CORES)))
    out = np.empty((B, T), np.float32)
    for i in range(N_CORES):
        O1 = np.asarray(res.results[i]["O1"]).astype(np.float32)  # [4, T*BH]
        for s in range(NSTREAM):
            bs = slice(i * B_local + s * BH, i * B_local + (s + 1) * BH)
            out[bs] = O1[s].reshape(T, BH).T
    return out
